# revision 35
# baseline (speedup 1.0000x reference)
"""CamLiPWC_Core fused kernel for Trainium2 (8 NeuronCores, SPMD).

Sharding: core c -> (b = c//4, quarter j = c%4).
  - correlation cost volume: 40-row h-slab of the [160,480] image per core
  - knn-interpolation / warp / uv: 2048 fine points per core
  - self-KNN max-aggregation: 512 coarse points per core
"""

import functools
import sys

for p in ("/opt/trn_rl_repo", "/root/.axon_site/_ro/trn_rl_repo"):
    if p not in sys.path:
        sys.path.insert(0, p)

import numpy as np

import concourse.bacc as bacc
import concourse.bass as bass
import concourse.mybir as mybir
from concourse.ap import AP
from concourse.bass import IndirectOffsetOnAxis
from concourse.tile import TileContext

F32 = mybir.dt.float32
F32R = mybir.dt.float32r
U32 = mybir.dt.uint32
ALU = mybir.AluOpType

# problem constants
B, N, M, C, H, W = 2, 8192, 2048, 64, 160, 480
FX = 1050.0
FY = 1050.0
CX = 479.5
CY = 269.5
SENSOR_H = 540.0
SENSOR_W = 960.0
SX = (W - 1) / (SENSOR_W - 1)
SY = (H - 1) / (SENSOR_H - 1)
K_SELF = 16
K_INTERP = 3
KCAND = 8  # approximate candidates rescored exactly

# sharding
NCORES = 8
HS = H // 4  # 40 h-rows per core
QS = N // 4  # 2048 fine points per core
MS = M // 4  # 512 coarse points per core

# correlation tiling
WBLK = 120
VW = WBLK + 8  # 128 moving window
NWB = W // WBLK  # 4
DYG = 3  # dy per matmul group -> N = 3*128 = 384
BANDF = 9 * VW  # 1152 restrided band width per h-row
PS_G = 512  # psum bank-aligned group stride (elements)
HPAIR = 4  # h-rows per band tile / diag DMA
QSLAB = 4  # h-rows per input slab load

FEATT_D = 72  # padded row: [xyz(3) | flow(3) | feat(64) | p2 | pad]

# 'act': leaky-relu fused into the ACT restride (hardware Prelu+alpha).
# 'dve': plain Copy restride + 2-op leaky-relu on DVE (CoreSim-safe).
LRELU_MODE = "act"


def build_nc():
    nc = bacc.Bacc(None, target_bir_lowering=False, debug=True)

    f1 = nc.declare_dram_parameter("f1", [C, HS, W], F32, isOutput=False)
    f2p = nc.declare_dram_parameter("f2p", [C, HS + 8, W + 8], F32, isOutput=False)
    xyzc = nc.declare_dram_parameter("xyzc", [3, M], F32, isOutput=False)
    x1q = nc.declare_dram_parameter("x1q", [3, QS], F32, isOutput=False)
    featc = nc.declare_dram_parameter("featc", [70, M], F32, isOutput=False)
    xqself = nc.declare_dram_parameter("xqself", [3, MS], F32, isOutput=False)

    corr_t = nc.declare_dram_parameter("corr_t", [HS, W, 81], F32, isOutput=True)
    interp_o = nc.declare_dram_parameter("interp_o", [67, QS], F32, isOutput=True)
    warp_o = nc.declare_dram_parameter("warp_o", [3, QS], F32, isOutput=True)
    uv_o = nc.declare_dram_parameter("uv_o", [2, QS], F32, isOutput=True)
    knnf_o = nc.declare_dram_parameter("knnf_o", [C, MS], F32, isOutput=True)

    featT = nc.dram_tensor("featT", [M, FEATT_D], F32)

    with TileContext(nc) as tc:
        with (
            tc.tile_pool(name="const", bufs=1) as cpool,
            tc.tile_pool(name="io", bufs=2) as iopool,
            tc.tile_pool(name="band", bufs=2) as bandpool,
            tc.tile_pool(name="scan", bufs=2) as scanpool,
            tc.tile_pool(name="gath", bufs=2) as gathpool,
            tc.tile_pool(name="small", bufs=3) as smallpool,
            tc.tile_pool(name="outb", bufs=1) as outpool,
            tc.tile_pool(name="cps", bufs=2, space="PSUM") as cpspool,
            tc.tile_pool(name="kps", bufs=2, space="PSUM") as kpspool,
        ):
            # ---------------- constants / prep ----------------
            ones = cpool.tile([128, 128], F32)
            nc.vector.memset(ones, 1.0)
            ident = cpool.tile([128, 128], F32)
            nc.gpsimd.affine_select(
                ident,
                ones,
                pattern=[[1, 128]],
                compare_op=ALU.is_equal,
                fill=0.0,
                base=0,
                channel_multiplier=-1,
            )

            onesrow = cpool.tile([1, 512], F32)
            nc.vector.memset(onesrow, 1.0)

            # keys4f = [x; y; z; -0.5*|p|^2] over full coarse set (fp32)
            keys4f = cpool.tile([4, M], F32)
            nc.sync.dma_start(out=keys4f[0:3, :], in_=xyzc[:, :])
            ones31 = cpool.tile([3, 1], F32)
            nc.vector.memset(ones31, 1.0)
            for ch in range(M // 512):
                sqc = smallpool.tile([3, 512], F32, tag="sqc", bufs=2, name=f"sqc{ch}")
                nc.vector.tensor_mul(
                    sqc,
                    keys4f[0:3, ch * 512 : (ch + 1) * 512],
                    keys4f[0:3, ch * 512 : (ch + 1) * 512],
                )
                p2ps = kpspool.tile([128, 512], F32, tag="kps", name=f"p2ps{ch}")
                nc.tensor.matmul(
                    p2ps[0:1, :], lhsT=ones31, rhs=sqc, start=True, stop=True
                )
                p2c = smallpool.tile([1, 512], F32, tag="p2c", name=f"p2c{ch}")
                nc.vector.tensor_scalar_mul(p2c, p2ps[0:1, :], -0.5)
                nc.sync.dma_start(
                    out=keys4f[3:4, ch * 512 : (ch + 1) * 512], in_=p2c
                )

            # fine-point queries (fp32 master + fp32r rounded copy)
            queries4 = cpool.tile([4, QS], F32)
            nc.sync.dma_start(out=queries4[0:3, :], in_=x1q[:, :])
            for ch in range(QS // 512):
                nc.sync.dma_start(
                    out=queries4[3:4, ch * 512 : (ch + 1) * 512],
                    in_=onesrow[0:1, :],
                )

            # self-KNN queries (exact fp32 scoring)
            qself4 = cpool.tile([4, MS], F32)
            nc.sync.dma_start(out=qself4[0:3, :], in_=xqself[:, :])
            nc.sync.dma_start(out=qself4[3:4, :], in_=onesrow[0:1, :])

            # featc70 = [xyz; flow; feat] -> transpose to featT [M, 72] in DRAM
            featc_sb = scanpool.tile([70, M], F32, tag="scan", bufs=3, name="featc_sb")
            nc.sync.dma_start(out=featc_sb, in_=featc[:, :])
            for t4 in range(M // 512):
                fts = smallpool.tile(
                    [128, 4 * 72], F32, tag="fts", bufs=2, name=f"fts{t4}"
                )
                for c4 in range(4):
                    t = t4 * 4 + c4
                    tp = kpspool.tile([128, 512], F32, tag="kps", name=f"ftp{t}")
                    nc.tensor.transpose(
                        tp[:, 0:70],
                        featc_sb[:, t * 128 : (t + 1) * 128],
                        ident[0:70, 0:70],
                    )
                    nc.vector.tensor_copy(
                        fts[:, c4 * 72 : c4 * 72 + 70], tp[:, 0:70]
                    )
                    nc.vector.memset(fts[:, c4 * 72 + 70 : c4 * 72 + 72], 0.0)
                # rows 512*t4 + 128*c4 + p  <- fts[p, c4*72 : +72]
                dst = AP(
                    featT,
                    512 * t4 * FEATT_D,
                    [[FEATT_D, 128], [128 * FEATT_D, 4], [1, FEATT_D]],
                )
                src = AP(fts.tensor, fts.offset, [[4 * 72, 128], [72, 4], [1, 72]])
                nc.sync.dma_start(out=dst, in_=src)

            # ---------------- correlation cost volume ----------------
            inv_c = 1.0 / C

            def corr_slab(sl):
                h0 = sl * QSLAB
                f1h = iopool.tile([C, QSLAB, W], F32R, tag="f1h", name=f"f1h{sl}")
                f2h = iopool.tile(
                    [C, QSLAB + 8, W + 8], F32R, tag="f2h", name=f"f2h{sl}"
                )
                nc.sync.dma_start(
                    out=f1h, in_=f1[:, h0 : h0 + QSLAB, :].bitcast(F32R)
                )
                nc.sync.dma_start(
                    out=f2h, in_=f2p[:, h0 : h0 + QSLAB + 8, :].bitcast(F32R)
                )
                for hp in range(QSLAB // HPAIR):
                    for wb in range(NWB):
                        w0 = wb * WBLK
                        band = bandpool.tile(
                            [128, HPAIR * BANDF],
                            F32,
                            tag="band",
                            name=f"bnd{sl}_{hp}_{wb}",
                        )
                        for r in range(HPAIR):
                            hl = hp * HPAIR + r
                            ps = cpspool.tile(
                                [128, 3 * PS_G],
                                F32,
                                tag="cps",
                                name=f"cps{sl}_{hp}_{wb}_{r}",
                            )
                            for g in range(DYG):
                                nc.tensor.matmul(
                                    ps[0:WBLK, PS_G * g : PS_G * g + DYG * VW],
                                    lhsT=f1h[:, hl, w0 : w0 + WBLK],
                                    rhs=f2h[
                                        :,
                                        hl + DYG * g : hl + DYG * g + DYG,
                                        w0 : w0 + VW,
                                    ],
                                    start=True,
                                    stop=True,
                                )
                            # ACT restride + leaky-relu + /C:
                            # band[p, r*1152 + u*9 + 3g + l] = act(ps[p, 512g+VW*l+u])
                            in_ap = AP(
                                ps.tensor,
                                ps.offset,
                                [[3 * PS_G, WBLK], [PS_G, 3], [VW, 3], [1, VW]],
                            )
                            out_ap = AP(
                                band.tensor,
                                band.offset + r * BANDF,
                                [[HPAIR * BANDF, WBLK], [3, 3], [1, 3], [9, VW]],
                            )
                            if LRELU_MODE == "act":
                                nc.scalar.activation(
                                    out_ap,
                                    in_ap,
                                    mybir.ActivationFunctionType.Prelu,
                                    scale=inv_c,
                                    alpha=0.1,
                                )
                            else:
                                nc.scalar.activation(
                                    out_ap,
                                    in_ap,
                                    mybir.ActivationFunctionType.Copy,
                                    scale=inv_c,
                                )
                        if LRELU_MODE != "act":
                            tsc = bandpool.tile(
                                [128, HPAIR * BANDF],
                                F32,
                                tag="tsc",
                                name=f"tsc{sl}_{hp}_{wb}",
                            )
                            nc.vector.tensor_scalar_mul(
                                tsc[0:WBLK, :], band[0:WBLK, :], 0.1
                            )
                            nc.vector.tensor_max(
                                band[0:WBLK, :], band[0:WBLK, :], tsc[0:WBLK, :]
                            )
                        # diagonal extraction DMA straight to DRAM:
                        # partition p covers [r*1152 + 9p, +81) for both rows
                        diag = AP(
                            band.tensor,
                            band.offset,
                            [[HPAIR * BANDF + 9, WBLK], [BANDF, HPAIR], [1, 81]],
                        )
                        dst = AP(
                            corr_t,
                            (h0 + hp * HPAIR) * W * 81 + w0 * 81,
                            [[81, WBLK], [W * 81, HPAIR], [1, 81]],
                        )
                        nc.sync.dma_start(out=dst, in_=diag)

            # batched whole-core output buffers
            big_out = outpool.tile([67, QS], F32)
            warp_sb2 = outpool.tile([3, QS], F32)
            uv_sb2 = outpool.tile([2, QS], F32)
            knnf_sb = outpool.tile([C, MS], F32)

            # ---------------- interp KNN (fine points) ----------------
            IGRP = 4

            def interp_group(t0):
                tiles = list(range(t0, t0 + IGRP))
                scans, v8s, i8s, g1s, tqs = {}, {}, {}, {}, {}
                for t in tiles:
                    qs = slice(t * 128, (t + 1) * 128)
                    scan = scanpool.tile([128, M], F32, tag="scan", bufs=3, name=f"iscan{t}")
                    for ch in range(M // 512):
                        dps = kpspool.tile(
                            [128, 512], F32, tag="kps", name=f"idps{t}_{ch}"
                        )
                        nc.tensor.matmul(
                            dps,
                            lhsT=queries4[:, qs],
                            rhs=keys4f[:, ch * 512 : (ch + 1) * 512],
                            start=True,
                            stop=True,
                        )
                        nc.vector.tensor_copy(
                            scan[:, ch * 512 : (ch + 1) * 512], dps
                        )
                    scans[t] = scan
                for t in tiles:
                    v8 = smallpool.tile([128, 8], F32, tag="v8", name=f"iv8{t}")
                    nc.vector.max(v8, scans[t])
                    i8 = smallpool.tile([128, 8], U32, tag="i8", bufs=6, name=f"ii8{t}")
                    nc.vector.max_index(i8, v8, scans[t])
                    v8s[t], i8s[t] = v8, i8
                for t in tiles:
                    g1 = gathpool.tile(
                        [128, KCAND * FEATT_D], F32, tag="g1", bufs=4, name=f"ig1{t}"
                    )
                    for k in range(KCAND):
                        nc.gpsimd.indirect_dma_start(
                            out=g1[:, k * FEATT_D : (k + 1) * FEATT_D],
                            out_offset=None,
                            in_=featT[:, :],
                            in_offset=IndirectOffsetOnAxis(
                                ap=i8s[t][:, k : k + 1], axis=0
                            ),
                        )
                    g1s[t] = g1
                for t in tiles:
                    qs = slice(t * 128, (t + 1) * 128)
                    tbndl = kpspool.tile([128, 512], F32, tag="kps", name=f"tb{t}")
                    nc.tensor.transpose(
                        tbndl[:, 0:3], queries4[0:3, qs], ident[0:3, 0:3]
                    )
                    tq = smallpool.tile([128, 3], F32, tag="tq", bufs=6, name=f"tq{t}")
                    nc.vector.tensor_copy(tq, tbndl[:, 0:3])
                    tqs[t] = (tq, tbndl)
                for t in tiles:
                    interp_finish(t, g1s[t], tqs[t])

            def interp_finish(t, g1, tq_bndl):
                qs = slice(t * 128, (t + 1) * 128)
                tq, tbndl = tq_bndl

                # exact candidate distances from gathered xyz
                dd = smallpool.tile([128, 3 * KCAND], F32, tag="dd", name=f"dd{t}")
                gx = AP(
                    g1.tensor,
                    g1.offset,
                    [[KCAND * FEATT_D, 128], [FEATT_D, KCAND], [1, 3]],
                )
                tqb = AP(tq.tensor, tq.offset, [[3, 128], [0, KCAND], [1, 3]])
                nc.vector.tensor_sub(dd, gx, tqb)
                nc.vector.tensor_mul(dd, dd, dd)
                d2 = smallpool.tile([128, KCAND], F32, tag="d2", name=f"d2{t}")
                dd3 = AP(
                    dd.tensor, dd.offset, [[3 * KCAND, 128], [3, KCAND], [1, 3]]
                )
                nc.vector.reduce_sum(d2, dd3, axis=mybir.AxisListType.X)

                # exact top-3 threshold: 3rd largest of -d2
                nd = smallpool.tile([128, KCAND], F32, tag="nd", name=f"nd{t}")
                nc.vector.tensor_scalar_mul(nd, d2, -1.0)
                nds = smallpool.tile([128, 8], F32, tag="nds", name=f"nds{t}")
                nc.vector.max(nds, nd)
                msk = smallpool.tile([128, KCAND], F32, tag="msk", name=f"msk{t}")
                nc.vector.tensor_scalar(
                    msk, nd, nds[:, 2:3], None, op0=ALU.is_ge
                )

                # inverse-distance weights, masked to the exact top-3
                dst_t = smallpool.tile([128, KCAND], F32, tag="dst", name=f"dst{t}")
                nc.scalar.activation(
                    dst_t, d2, mybir.ActivationFunctionType.Sqrt, scale=1.0
                )
                nc.vector.tensor_scalar_add(dst_t, dst_t, 1e-8)
                wts = smallpool.tile([128, KCAND], F32, tag="wts", name=f"wts{t}")
                nc.vector.reciprocal(wts, dst_t)
                nc.vector.tensor_mul(wts, wts, msk)
                wsum = smallpool.tile([128, 1], F32, tag="wsum", name=f"wsum{t}")
                nc.vector.reduce_sum(wsum, wts, axis=mybir.AxisListType.X)
                wsi = smallpool.tile([128, 1], F32, tag="wsi", name=f"wsi{t}")
                nc.vector.reciprocal(wsi, wsum)
                nc.vector.tensor_scalar(wts, wts, wsi, None, op0=ALU.mult)

                # weighted combine over 8 candidates (masked): 2 strided ops
                prod = smallpool.tile(
                    [128, 67 * KCAND], F32, tag="prod", bufs=2, name=f"prod{t}"
                )
                gfeat = AP(
                    g1.tensor,
                    g1.offset + 3,
                    [[KCAND * FEATT_D, 128], [1, 67], [FEATT_D, KCAND]],
                )
                wb_ap = AP(
                    wts.tensor, wts.offset, [[KCAND, 128], [0, 67], [1, KCAND]]
                )
                nc.vector.tensor_mul(prod, gfeat, wb_ap)
                acc = smallpool.tile([128, 67], F32, tag="acc", name=f"acc{t}")
                prod3 = AP(
                    prod.tensor,
                    prod.offset,
                    [[67 * KCAND, 128], [KCAND, 67], [1, KCAND]],
                )
                nc.vector.reduce_sum(acc, prod3, axis=mybir.AxisListType.X)

                # transpose interp back to channel-major into the out buffer
                itp = kpspool.tile([128, 512], F32, tag="kps", name=f"itp{t}")
                nc.tensor.transpose(itp[0:67, 0:128], acc, ident[:, :])
                nc.vector.tensor_copy(big_out[0:67, qs], itp[0:67, 0:128])

                # warp = xyz1 + interp[:3]
                nc.vector.tensor_add(
                    warp_sb2[0:3, qs], itp[0:3, 0:128], queries4[0:3, qs]
                )

                # uv projection (q-major then transpose)
                zi = smallpool.tile([128, 1], F32, tag="zi", name=f"zi{t}")
                nc.vector.reciprocal(zi, tq[:, 2:3])
                uvq = smallpool.tile([128, 2], F32, tag="uvq", name=f"uvq{t}")
                xz = smallpool.tile([128, 1], F32, tag="xz", name=f"xz{t}")
                nc.vector.tensor_mul(xz, tq[:, 0:1], zi)
                nc.vector.tensor_scalar(
                    uvq[:, 0:1], xz, FX * SX, CX * SX, op0=ALU.mult, op1=ALU.add
                )
                nc.vector.tensor_mul(xz, tq[:, 1:2], zi)
                nc.vector.tensor_scalar(
                    uvq[:, 1:2], xz, FY * SY, CY * SY, op0=ALU.mult, op1=ALU.add
                )
                nc.tensor.transpose(itp[0:2, 128:256], uvq, ident[:, :])
                nc.vector.tensor_copy(uv_sb2[0:2, qs], itp[0:2, 128:256])

            # ---------------- self-KNN max aggregation ----------------
            def self_tile(t):
                ms = slice(t * 128, (t + 1) * 128)
                scan = scanpool.tile([128, M], F32, tag="scan", bufs=3, name=f"sscan{t}")
                for ch in range(M // 512):
                    dps = kpspool.tile([128, 512], F32, tag="kps", name=f"sdps{t}_{ch}")
                    nc.tensor.matmul(
                        dps,
                        lhsT=qself4[:, ms],
                        rhs=keys4f[:, ch * 512 : (ch + 1) * 512],
                        start=True,
                        stop=True,
                    )
                    nc.vector.tensor_copy(scan[:, ch * 512 : (ch + 1) * 512], dps)
                v8a = smallpool.tile([128, 8], F32, tag="v8", name=f"sv8a{t}")
                nc.vector.max(v8a, scan)
                i16 = smallpool.tile([128, 16], U32, tag="i16", name=f"si16{t}")
                nc.vector.max_index(i16[:, 0:8], v8a, scan)
                mr = scanpool.tile([128, M], F32, tag="scan", bufs=3, name=f"smr{t}")
                nc.vector.match_replace(mr, v8a, scan, -1e30)
                v8b = smallpool.tile([128, 8], F32, tag="v8b", name=f"sv8b{t}")
                nc.vector.max(v8b, mr)
                nc.vector.max_index(i16[:, 8:16], v8b, mr)

                g2 = gathpool.tile(
                    [128, K_SELF * FEATT_D], F32, tag="g2", name=f"sg2{t}"
                )
                for k in range(K_SELF):
                    nc.gpsimd.indirect_dma_start(
                        out=g2[:, k * FEATT_D : (k + 1) * FEATT_D],
                        out_offset=None,
                        in_=featT[:, :],
                        in_offset=IndirectOffsetOnAxis(
                            ap=i16[:, k : k + 1], axis=0
                        ),
                    )
                kf = smallpool.tile([128, C], F32, tag="kf", name=f"kf{t}")
                gfeat = AP(
                    g2.tensor,
                    g2.offset + 6,
                    [[K_SELF * FEATT_D, 128], [1, C], [FEATT_D, K_SELF]],
                )
                nc.vector.tensor_reduce(
                    out=kf, in_=gfeat, op=ALU.max, axis=mybir.AxisListType.X
                )
                kfp = kpspool.tile([128, 512], F32, tag="kps", name=f"kfp{t}")
                nc.tensor.transpose(kfp[0:C, 0:128], kf, ident[:, :])
                nc.vector.tensor_copy(knnf_sb[:, ms], kfp[0:C, 0:128])

            # ---------------- interleaved emission ----------------
            nslab = HS // QSLAB
            ngrp = QS // 128 // IGRP
            nst = MS // 128
            ig_done = 0
            st_done = 0
            for sl in range(nslab):
                corr_slab(sl)
                ig_target = (sl + 1) * ngrp // nslab
                while ig_done < ig_target:
                    interp_group(ig_done * IGRP)
                    ig_done += 1
                st_target = (sl + 1) * nst // nslab
                while st_done < st_target:
                    self_tile(st_done)
                    st_done += 1

            nc.sync.dma_start(out=interp_o[:, :], in_=big_out[:, :])
            nc.sync.dma_start(out=warp_o[:, :], in_=warp_sb2[0:3, :])
            nc.sync.dma_start(out=uv_o[:, :], in_=uv_sb2[0:2, :])
            nc.sync.dma_start(out=knnf_o[:, :], in_=knnf_sb)

    nc.compile()
    return nc


@functools.cache
def _get_nc():
    return build_nc()


def _shard_inputs(xyz1, xyz_coarse, flow_coarse, feat_coarse, feat1_2d, feat2_2d):
    """Build the 8 per-core input maps (host-side slicing/padding only)."""
    in_maps = []
    for c in range(NCORES):
        b, j = divmod(c, 4)
        h0 = j * HS
        f1s = np.ascontiguousarray(feat1_2d[b, :, h0 : h0 + HS, :])
        # padded f2 slab: rows [h0-4, h0+HS+4), cols padded by 4 each side
        f2s = np.zeros((C, HS + 8, W + 8), dtype=np.float32)
        r0, r1 = max(0, h0 - 4), min(H, h0 + HS + 4)
        f2s[:, r0 - (h0 - 4) : r1 - (h0 - 4), 4 : 4 + W] = feat2_2d[b, :, r0:r1, :]
        featc70 = np.concatenate(
            [xyz_coarse[b], flow_coarse[b], feat_coarse[b]], axis=0
        ).astype(np.float32)
        in_maps.append(
            {
                "f1": f1s,
                "f2p": f2s,
                "xyzc": np.ascontiguousarray(xyz_coarse[b]),
                "x1q": np.ascontiguousarray(xyz1[b, :, j * QS : (j + 1) * QS]),
                "featc": featc70,
                "xqself": np.ascontiguousarray(
                    xyz_coarse[b, :, j * MS : (j + 1) * MS]
                ),
            }
        )
    return in_maps


def _assemble(results):
    """Gather the 8 per-core result dicts into full-shape outputs."""
    interp = np.zeros((B, 67, N), dtype=np.float32)
    warp = np.zeros((B, 3, N), dtype=np.float32)
    uv = np.zeros((B, 2, N), dtype=np.float32)
    knnf = np.zeros((B, C, M), dtype=np.float32)
    corr = np.zeros((B, 81, H, W), dtype=np.float32)
    for c in range(NCORES):
        b, j = divmod(c, 4)
        r = results[c]
        interp[b, :, j * QS : (j + 1) * QS] = r["interp_o"]
        warp[b, :, j * QS : (j + 1) * QS] = r["warp_o"]
        uv[b, :, j * QS : (j + 1) * QS] = r["uv_o"]
        knnf[b, :, j * MS : (j + 1) * MS] = r["knnf_o"]
        # corr_t layout [h, w, dx, dy] -> corr [s=dy*9+dx, h, w]
        ct = r["corr_t"].reshape(HS, W, 9, 9)
        corr[b, :, j * HS : (j + 1) * HS, :] = (
            ct.transpose(3, 2, 0, 1).reshape(81, HS, W)
        )
    return interp, warp, uv, knnf, corr


def kernel(xyz1, xyz_coarse, flow_coarse, feat_coarse, feat1_2d, feat2_2d):
    from concourse.bass_utils import run_bass_kernel_spmd

    nc = _get_nc()
    in_maps = _shard_inputs(
        xyz1, xyz_coarse, flow_coarse, feat_coarse, feat1_2d, feat2_2d
    )
    res = run_bass_kernel_spmd(nc, in_maps, list(range(NCORES)))
    return _assemble(res.results)
